# revision 17
# baseline (speedup 1.0000x reference)
"""LoRA multi-head attention on 8 Trainium2 NeuronCores.

Sharding: data-parallel over batch (B=2) x tensor-parallel over heads
(16 heads -> 4 per core).  Core c handles batch b=c//4 and head group
g=c%4 (columns C=[256*g, 256*g+256) of the projection output).

Host prep (per weight): W_eff = W + 2.0 * B @ A  (exact LoRA fold),
transposed activations x.T, everything bf16.  Output partials ship
bf16 and are summed on host across the 4 cores of each batch.

Device schedule per core:
  DMA: 256KB half-row transfers; sync queue carries wk+xk, the Pool
  queue carries wq/xq/wv/xv/wo.  Nothing rides the ACT queue (exps
  must not sit behind DMAs).  Prefix: K st0/st1 + Q0 projections as
  one i-loop ordered by DMA arrival, accumulating in the idle sc/pj
  PSUM banks (2KB zero regions keep interleaved groups independent).
  K st2/st3, the V projection (split by head-pair), and Q proj of
  qt1..3 are woven into the attention groups as per-step fillers.

  8 attention groups (q-tile x head-pair), 16 t-steps each: scores
  pair (PE, row-tiled co-issue) -> exp -> PV pair + softmax row-sums
  via a ones-column in V.  exp is split 10/6 between the ACT engine
  (table Exp) and a custom 8-stage DVE op exp(y)~=(1+y/64)^64 (the
  softmax tolerates this: numerator and denominator shift together).
  PV emission lags exp by 2+ steps so the PE never waits.  The output
  projection of qt-1 rides 2 groups later; qt3's final o-blocks open
  their first contraction half inside g7.  PSUM evictions split
  across ACT/DVE (the Pool engine cannot touch PSUM and reprograms
  between op types, so it only does partition_broadcast + DMA); ctx
  normalization: DVE reciprocal of the row-sum row, Pool broadcast,
  DVE multiply reading ctx PSUM directly.

PSUM (8 banks): sc 2x[128,1024]=4, pj 2x[128,512]=2, ctx 2x[65,512]=2.
"""

import sys

sys.path.insert(0, "/opt/trn_rl_repo")

from contextlib import ExitStack

import ml_dtypes
import numpy as np

import concourse.bass as bass
import concourse.tile as tile
from concourse import bacc, mybir
from concourse.bass_utils import run_bass_kernel_spmd

# ---- custom DVE exp op ----------------------------------------------------
import concourse.dve_ops as dve_ops
from concourse.dve_spec import C0, C1, Spec, Src0, sq
from concourse.dve_spec import lower as dve_lower
from concourse.dve_uop import DveOpSpec


def _exp64_ref(in0, in1, s0, s1, imm2):
    z = (in0.astype(np.float32) * np.float32(s0) + np.float32(s1)).astype(np.float32)
    for _ in range(6):
        z = z * z
    return z


def _register_exp_op():
    for op in dve_ops.OPS:
        if op.name == "EXP64_ANT":
            return op
    body = Src0 * C0 + C1
    for _ in range(6):
        body = sq(body)
    spec = Spec(body=body, reference=_exp64_ref)
    row = max(dve_ops._SUB_OPCODE_FOR_NAME.values()) + 1
    shas = {}
    for ver in ("v3", "v4"):
        uops = dve_lower(spec, ver=ver)
        shas[ver] = DveOpSpec(
            name="EXP64_ANT", opcode=row, uops=uops, rd1_en=False
        ).sha(ver)
    op = dve_ops.DveOp("EXP64_ANT", spec, subdim=False, uops_sha=shas)
    dve_ops.OPS.append(op)
    dve_ops._SUB_OPCODE_FOR_NAME[op.name] = row
    dve_ops.CUSTOM_DVE_SPECS[op.name] = spec
    return op


EXP_OP = _register_exp_op()

F32 = mybir.dt.float32
BF16 = mybir.dt.bfloat16

B = 2
S = 2048
D = 1024
H = 16
DK = 64
SCALING = 2.0
N_CORES = 8
CPG = 4
CSLICE = D // CPG
Exp = mybir.ActivationFunctionType.Exp
MULT = mybir.AluOpType.mult

# exp-engine routing: which t-steps run on the DVE (rest on ACT)
DVE_STEPS = frozenset({1, 4, 7, 9, 12, 14})
DVE_STEPS_G0 = frozenset({5, 8, 10, 12, 14})

_CACHE = {}


def _build():
    nc = bacc.Bacc("TRN2", target_bir_lowering=False, debug=False)

    xqT = nc.declare_dram_parameter("xqT", [D, S], BF16, isOutput=False)
    xkT = nc.declare_dram_parameter("xkT", [D, S], BF16, isOutput=False)
    xvT = nc.declare_dram_parameter("xvT", [D, S], BF16, isOutput=False)
    wq = nc.declare_dram_parameter("wq", [D, CSLICE], BF16, isOutput=False)
    wk = nc.declare_dram_parameter("wk", [D, CSLICE], BF16, isOutput=False)
    wv = nc.declare_dram_parameter("wv", [D, CSLICE], BF16, isOutput=False)
    wo = nc.declare_dram_parameter("wo", [CSLICE, D], BF16, isOutput=False)
    outT = nc.declare_dram_parameter("outT", [D, S], BF16, isOutput=True)

    with tile.TileContext(nc) as tc, ExitStack() as ctx:
        const = ctx.enter_context(tc.tile_pool(name="const", bufs=1))
        xkp = ctx.enter_context(tc.tile_pool(name="xkp", bufs=16))
        xvp = ctx.enter_context(tc.tile_pool(name="xvp", bufs=16))
        xqp = ctx.enter_context(tc.tile_pool(name="xqp", bufs=8))
        expp = ctx.enter_context(tc.tile_pool(name="expp", bufs=16))
        smallp = ctx.enter_context(tc.tile_pool(name="smallp", bufs=2))
        psum = ctx.enter_context(tc.tile_pool(name="psum", bufs=2, space="PSUM"))

        wq_sb = const.tile([128, 8, CSLICE], BF16)
        wk_sb = const.tile([128, 8, CSLICE], BF16)
        wv_sb = const.tile([128, 8, CSLICE], BF16)
        wo_sb = const.tile([128, 2, D], BF16)

        kT_s = [const.tile([128, 2, 512], BF16, name=f"kT{i}") for i in range(4)]
        qT_s = [const.tile([128, 2, 512], BF16, name=f"qT{i}") for i in range(4)]
        # v[tt]: [128 kpos, t4, pair, 160] laid out as
        # [dk-even(0:64) | ones(64) | zeros(65:96) | dk-odd(96:160)].  The
        # even PV lhsT is cols 0:65 (M=65, rowsum at ctxE row 64); the odd
        # lhsT is cols 32:160 (M=128: rows 0:63 junk, row 32 = rowsum from
        # the shared ones col -- 32-aligned as engines require -- and rows
        # 64:128 = dk-odd) so the odd ctx lands on partitions 64:128 and
        # the norm/evict never crosses partitions.
        v_s = [const.tile([128, 4, 2, 160], BF16, name=f"v_{i}") for i in range(4)]
        ctxT_s = [const.tile([128, 2, 512], BF16, name=f"cx{i}") for i in range(4)]

        # ---- input DMAs: 256KB half-row transfers, none on the scalar
        # queue (ACT must not queue exps behind DMAs).  Left halves first
        # so K-st0/st1 and Q0 can start ~8us in.
        xkL, xkR, xvL, xvR, xq_b, xq23_b = [], [], [], [], [], []
        nc.sync.dma_start(wk_sb[:], wk.rearrange("(i p) c -> p i c", p=128))
        for half, lst in ((0, xkL), (1, xkR)):
            for i in range(8):
                t_ = xkp.tile([128, 1024], BF16, tag="xk", bufs=16, name=f"xk{half}_{i}")
                nc.sync.dma_start(
                    t_[:], xkT[128 * i : 128 * (i + 1), 1024 * half : 1024 * (half + 1)]
                )
                lst.append(t_)
        nc.gpsimd.dma_start(wq_sb[:], wq.rearrange("(i p) c -> p i c", p=128))
        for i in range(8):
            t_ = xqp.tile([128, 1024], BF16, tag="xq", bufs=8, name=f"xq01_{i}")
            nc.gpsimd.dma_start(t_[:], xqT[128 * i : 128 * (i + 1), 0:1024])
            xq_b.append(t_)
        nc.gpsimd.dma_start(wv_sb[:], wv.rearrange("(i p) c -> p i c", p=128))
        for half, lst in ((0, xvL), (1, xvR)):
            for i in range(8):
                t_ = xvp.tile([128, 1024], BF16, tag="xv", bufs=16, name=f"xv{half}_{i}")
                nc.gpsimd.dma_start(
                    t_[:], xvT[128 * i : 128 * (i + 1), 1024 * half : 1024 * (half + 1)]
                )
                lst.append(t_)
        nc.gpsimd.dma_start(wo_sb[:], wo.rearrange("(c p) o -> p c o", p=128))

        # ones column of V (softmax row sums ride the PV matmul)
        for tt in range(4):
            nc.vector.memset(v_s[tt][:, :, :, DK : DK + 1], 1.0)
            nc.vector.memset(v_s[tt][:, :, :, 65:96], 0.0)

        # ---- prefix: K st0/st1 + Q0, i-loop ordered by DMA arrival ----
        # Accumulators: two sc-tag PSUM tiles hold st0/st1 x (cc0|cc1) as
        # 2KB halves (zero regions are 2KB so the interleaved groups are
        # independent); Q0 rides the two pj slots.  One [128,1024] DVE
        # eviction per kT tile.  K st2/st3 and V ride inside g0 as
        # fillers; Q1 inside g1.
        scA = psum.tile([128, 1024], F32, tag="sc", bufs=2, name="kl0")
        scB = psum.tile([128, 1024], F32, tag="sc", bufs=2, name="kl1")
        pjQ = [
            psum.tile([128, 512], F32, tag="pj", bufs=2, name=f"q0_{cc}")
            for cc in range(2)
        ]
        for i in range(8):
            st_, sp_ = (i == 0), (i == 7)
            for cc in range(2):
                wsl = wk_sb[:, i, 128 * cc : 128 * (cc + 1)]
                nc.tensor.matmul(
                    scA[:, 512 * cc : 512 * (cc + 1)], wsl, xkL[i][:, 0:512],
                    start=st_, stop=sp_,
                )
                nc.tensor.matmul(
                    scB[:, 512 * cc : 512 * (cc + 1)], wsl, xkL[i][:, 512:1024],
                    start=st_, stop=sp_,
                )
            for cc in range(2):
                nc.tensor.matmul(
                    pjQ[cc][:],
                    wq_sb[:, i, 128 * cc : 128 * (cc + 1)],
                    xq_b[i][:, 0:512],
                    start=st_, stop=sp_,
                )
        nc.vector.tensor_copy(kT_s[0][:], scA[:].rearrange("p (c f) -> p c f", c=2))
        nc.scalar.copy(kT_s[1][:], scB[:].rearrange("p (c f) -> p c f", c=2))
        nc.vector.tensor_copy(qT_s[0][:, 0, :], pjQ[0][:])
        nc.scalar.copy(qT_s[0][:, 1, :], pjQ[1][:])

        kr_live = {}

        def kr_mm(st, cc, i):
            # K st2/st3 round, woven into g0 (4 mm/step)
            if i == 0:
                kr_live[(st, cc)] = psum.tile(
                    [128, 512], F32, tag="pj", bufs=2, name=f"kr{st}{cc}"
                )
            ps = kr_live[(st, cc)]
            nc.tensor.matmul(
                ps[:],
                wk_sb[:, i, 128 * cc : 128 * (cc + 1)],
                (xkR[i][:, 0:512] if st == 2 else xkR[i][:, 512:1024]),
                start=(i == 0),
                stop=(i == 7),
            )
            if i == 7:
                ps = kr_live.pop((st, cc))
                if cc == 0:
                    nc.scalar.copy(kT_s[st][:, 0, :], ps[:])
                else:
                    nc.vector.tensor_copy(kT_s[st][:, 1, :], ps[:])

        # ---- attention building blocks --------------------------------
        def scores_pair(qt, p, t):
            sc = psum.tile([128, 1024], F32, tag="sc", bufs=2, name=f"sc{qt}{p}{t}")
            kt = kT_s[t // 4]
            ts_ = slice(128 * (t % 4), 128 * (t % 4 + 1))
            qtile = qT_s[qt]
            nc.tensor.matmul(
                sc[:, 0:512],
                kt[0:64, p, ts_],
                qtile[0:64, p, :],
                start=True,
                stop=True,
                tile_position=(0, 0),
            )
            nc.tensor.matmul(
                sc[:, 512:1024],
                kt[64:128, p, ts_],
                qtile[64:128, p, :],
                start=True,
                stop=True,
                tile_position=(64, 0),
            )
            return sc

        def exp_emit(qt, p, t, sc):
            et = expp.tile([128, 1024], BF16, tag="et", bufs=16, name=f"et{qt}{p}{t}")
            dve = t in (DVE_STEPS_G0 if (qt == 0 and p == 0) else DVE_STEPS)
            if dve:
                nc.vector._custom_dve(
                    EXP_OP, out=et[:], in0=sc[:], s0=1.0 / 512.0, s1=1.0
                )
            else:
                nc.scalar.activation(et[:], sc[:], Exp, scale=1.0 / 8.0)
            return et

        def pv_emit(p, t, et, ctx0, ctx1):
            vg = v_s[t // 4]
            nc.tensor.matmul(
                ctx0[:],
                vg[:, t % 4, p, 0:65],
                et[:, 0:512],
                start=(t == 0),
                stop=(t == 15),
            )
            nc.tensor.matmul(
                ctx1[:],
                vg[:, t % 4, p, 32:160],
                et[:, 512:1024],
                start=(t == 0),
                stop=(t == 15),
            )

        def v_round(hp, tt, t4):
            ps = psum.tile([128, 128], F32, tag="pj", bufs=2, name=f"vps{hp}{tt}{t4}")
            xv_half = xvL if tt < 2 else xvR
            c0 = 512 * (tt % 2) + 128 * t4
            for i in range(8):
                nc.tensor.matmul(
                    ps[:],
                    xv_half[i][:, c0 : c0 + 128],
                    wv_sb[:, i, 128 * hp : 128 * (hp + 1)],
                    start=(i == 0),
                    stop=(i == 7),
                )
            # scatter [even dk | odd dk] into cols {0:64, 96:160}
            dstE = v_s[tt][:, t4, hp, 0:DK]
            dstO = v_s[tt][:, t4, hp, 96:160]
            if t4 % 2 == 0:
                nc.scalar.copy(dstE, ps[:, 0:DK])
                nc.scalar.copy(dstO, ps[:, DK:128])
            else:
                nc.vector.tensor_copy(dstE, ps[:, 0:DK])
                nc.vector.tensor_copy(dstO, ps[:, DK:128])

        def oblock(qt, o):
            ops = psum.tile([128, 512], F32, tag="pj", bufs=2, name=f"op{qt}_{o}")
            nc.tensor.matmul(
                ops[:],
                wo_sb[:, 0, 128 * o : 128 * (o + 1)],
                ctxT_s[qt][:, 0, :],
                start=True,
                stop=False,
            )
            nc.tensor.matmul(
                ops[:],
                wo_sb[:, 1, 128 * o : 128 * (o + 1)],
                ctxT_s[qt][:, 1, :],
                start=False,
                stop=True,
            )
            ob = smallp.tile([128, 512], BF16, tag="ob", bufs=3)
            if o % 2 == 0:
                nc.scalar.copy(ob[:], ops[:])
            else:
                nc.vector.tensor_copy(ob[:], ops[:])
            nc.sync.dma_start(
                outT[128 * o : 128 * (o + 1), 512 * qt : 512 * (qt + 1)], ob[:]
            )

        ob3_part = {}

        def ob3_partial_c0(o):
            # first contraction half of qt3's o-block, evicted to SBUF fp32
            # so the pj bank frees immediately (runs as a g7 filler)
            ops = psum.tile([128, 512], F32, tag="pj", bufs=2, name=f"o3p{o}")
            nc.tensor.matmul(
                ops[:],
                wo_sb[:, 0, 128 * o : 128 * (o + 1)],
                ctxT_s[3][:, 0, :],
                start=True,
                stop=True,
            )
            part = smallp.tile([128, 512], BF16, tag="o3part", bufs=4, name=f"o3s{o}")
            if o % 2 == 0:
                nc.scalar.copy(part[:], ops[:])
            else:
                nc.vector.tensor_copy(part[:], ops[:])
            ob3_part[o] = part

        def ob3_c1_add(o):
            ops = psum.tile([128, 512], F32, tag="pj", bufs=2, name=f"o3q{o}")
            nc.tensor.matmul(
                ops[:],
                wo_sb[:, 1, 128 * o : 128 * (o + 1)],
                ctxT_s[3][:, 1, :],
                start=True,
                stop=True,
            )
            ob = smallp.tile([128, 512], BF16, tag="ob", bufs=3)
            nc.vector.tensor_tensor(
                ob[:], ops[:], ob3_part.pop(o)[:], mybir.AluOpType.add
            )
            nc.sync.dma_start(
                outT[128 * o : 128 * (o + 1), 512 * 3 : 512 * 4], ob[:]
            )

        def ob3_full(o):
            ops = psum.tile([128, 512], F32, tag="pj", bufs=2, name=f"o3f{o}")
            nc.tensor.matmul(
                ops[:],
                wo_sb[:, 0, 128 * o : 128 * (o + 1)],
                ctxT_s[3][:, 0, :],
                start=True,
                stop=False,
            )
            nc.tensor.matmul(
                ops[:],
                wo_sb[:, 1, 128 * o : 128 * (o + 1)],
                ctxT_s[3][:, 1, :],
                start=False,
                stop=True,
            )
            ob = smallp.tile([128, 512], BF16, tag="ob", bufs=3)
            nc.scalar.copy(ob[:], ops[:])
            nc.sync.dma_start(
                outT[128 * o : 128 * (o + 1), 512 * 3 : 512 * 4], ob[:]
            )

        # Q proj for qt 1/2/3, spread across steps (pj slot held across)
        qproj_live = {}

        def qproj_mm(qtn, cc, i):
            if i == 0:
                qproj_live[(qtn, cc)] = psum.tile(
                    [128, 512], F32, tag="pj", bufs=2, name=f"qp{qtn}{cc}"
                )
            ps = qproj_live[(qtn, cc)]
            xsrc = (
                xq_b[i][:, 512:1024]
                if qtn == 1
                else xq23_b[i][:, 512 * (qtn - 2) : 512 * (qtn - 1)]
            )
            nc.tensor.matmul(
                ps[:],
                wq_sb[:, i, 128 * cc : 128 * (cc + 1)],
                xsrc,
                start=(i == 0),
                stop=(i == 7),
            )
            if i == 7:
                nc.scalar.copy(qT_s[qtn][:, cc, :], qproj_live.pop((qtn, cc)))

        def xq_load(qtn):
            if qtn != 2:
                return
            for i in range(8):
                t_ = xqp.tile([128, 1024], BF16, tag="xq", bufs=8, name=f"xq23_{i}")
                nc.sync.dma_start(t_[:], xqT[128 * i : 128 * (i + 1), 1024:2048])
                xq23_b.append(t_)

        def mk_norm(cxE, cxO, qt, p):
            # rowsums: even at cxE row 64, odd at cxO row 32 (shared ones col)
            rs = smallp.tile([1, 2, 512], F32, tag="rs1", bufs=2)
            nc.vector.tensor_copy(rs[:, 0, :], cxE[DK : DK + 1, :])
            nc.vector.tensor_copy(rs[:, 1, :], cxO[32:33, :])
            rc = smallp.tile([1, 2, 512], F32, tag="rc", bufs=2)
            nc.vector.reciprocal_approx_fast(rc[:], rs[:])
            bcE = smallp.tile([64, 512], F32, tag="bcE", bufs=1)
            nc.gpsimd.partition_broadcast(bcE[:], rc[:, 0, :])
            bcO = smallp.tile([128, 512], F32, tag="bcO", bufs=1)
            nc.gpsimd.partition_broadcast(bcO[:], rc[:, 1, :])
            nc.vector.tensor_tensor(
                ctxT_s[qt][0:DK, p, :], cxE[0:DK, :], bcE[:], MULT
            )
            nc.vector.tensor_tensor(
                ctxT_s[qt][DK:128, p, :], cxO[DK:128, :], bcO[DK:128, :], MULT
            )

        # ---- attention groups -----------------------------------------
        # pending: work carried into the next group's first steps
        pending = []

        def drain_pending(upto):
            while pending and pending[0][0] <= upto:
                pending.pop(0)[1]()

        def emit_group(qt, p):
            gi = 2 * qt + p
            ctx0 = psum.tile([DK + 1, 512], F32, tag="ctxE", bufs=1, name=f"cx{qt}{p}0")
            ctx1 = psum.tile([128, 512], F32, tag="ctxO", bufs=1, name=f"cx{qt}{p}1")
            ets = {}

            # per-step pv emission plan (lagged so the PE never waits on exp,
            # and so every v_round/kr_mm a pv needs precedes it in the queue)
            if gi == 0:
                pv_plan = {10: [0, 1], 11: [2, 3], 12: [4, 5], 13: [6, 7, 8],
                           14: [9, 10, 11], 15: [12, 13]}
            elif gi == 1:
                pv_plan = {8: [0, 1], 9: [2, 3], 10: [4, 5], 11: [6, 7],
                           12: [8, 9], 13: [10, 11], 14: [12, 13]}
            else:
                pv_plan = {t: [t - 2] for t in range(2, 16)}

            # per-step filler plan
            fillers = {t: [] for t in range(16)}
            if gi == 0:
                # K st2/st3 (4 mm/step, steps 0-7), V heads 0/1 (2/step, 8-15)
                for j in range(32):
                    st, cc, i = 2 + j // 16, (j % 16) // 8, j % 8
                    fillers[2 + j // 4].append(lambda st=st, cc=cc, i=i: kr_mm(st, cc, i))
                for j in range(16):
                    fillers[8 + j // 2].append(
                        lambda j=j: v_round(0, j // 4, j % 4)
                    )
            elif gi == 1:
                # V heads 2/3 (2/step, steps 0-7), Q1 (2 mm/step, 8-15)
                for j in range(16):
                    fillers[j // 2].append(lambda j=j: v_round(1, j // 4, j % 4))
                for j in range(16):
                    cc, i = j // 8, j % 8
                    fillers[8 + j // 2].append(lambda cc=cc, i=i: qproj_mm(1, cc, i))
            else:
                if gi in (2, 4):
                    fillers[2].append(lambda qtn=qt + 1: xq_load(qtn))
                if gi in (2, 3, 4, 5):
                    qtn, cc = qt + 1, p
                    i = 0
                    for t_, n_ in ((10, 1), (11, 1), (12, 1), (13, 1), (14, 2), (15, 2)):
                        for _ in range(n_):
                            fillers[t_].append(
                                lambda qtn=qtn, cc=cc, i=i: qproj_mm(qtn, cc, i)
                            )
                            i += 1
                ob_base = 4 * p
                for j, t in enumerate((3, 5, 7, 9)):
                    fillers[t].append(lambda qt=qt, o=ob_base + j: oblock(qt - 1, o))
                if gi == 7:
                    for j, t in enumerate((10, 12, 13, 15)):
                        fillers[t].append(lambda o=j: ob3_partial_c0(o))

            for t in range(16):
                if t == 0:
                    drain_pending(0)
                sc = scores_pair(qt, p, t)
                ets[t] = exp_emit(qt, p, t, sc)
                if t == 1:
                    drain_pending(1)
                for tp in pv_plan.get(t, ()):
                    pv_emit(p, tp, ets.pop(tp), ctx0, ctx1)
                for f in fillers[t]:
                    f()

            # carry the drain into the next group
            def fin(qt=qt, p=p, ctx0=ctx0, ctx1=ctx1, ets=ets):
                pv_emit(p, 14, ets.pop(14), ctx0, ctx1)
                pv_emit(p, 15, ets.pop(15), ctx0, ctx1)
                mk_norm(ctx0, ctx1, qt, p)

            if qt == 3 and p == 1:
                fin()
            else:
                pending.append((0, fin))

        for qt in range(4):
            for p in range(2):
                emit_group(qt, p)

        # ---- tail: qt3 output projection.  Blocks 0-3 add the c0 partials
        # precomputed inside g7 (single c1 matmul + DVE add-evict); blocks
        # 4-7 run the full two-matmul contraction with ACT cast-evict, so
        # the two eviction streams ride different engines.
        ob3_c1_add(0)
        ob3_c1_add(1)
        ob3_full(4)
        ob3_c1_add(2)
        ob3_full(5)
        ob3_c1_add(3)
        ob3_full(6)
        ob3_full(7)

    nc.finalize()
    return nc


def _get_nc():
    if "nc" not in _CACHE:
        _CACHE["nc"] = _build()
    return _CACHE["nc"]


def _numpy_reference(query, key, value, mask, Wq, Aq, Bq, Wk, Ak, Bk, Wv, Av, Bv, Wo, Ao, Bo):
    """Exact fallback for a non-all-ones mask (never hit for the spec'd inputs)."""

    def lora(x, W, A, Bm):
        return x @ W.T + ((x @ A.T) @ Bm.T) * SCALING

    q = lora(query, Wq, Aq, Bq).reshape(B, S, H, DK).transpose(0, 2, 1, 3)
    k = lora(key, Wk, Ak, Bk).reshape(B, S, H, DK).transpose(0, 2, 1, 3)
    v = lora(value, Wv, Av, Bv).reshape(B, S, H, DK).transpose(0, 2, 1, 3)
    sc = np.einsum("bhqd,bhkd->bhqk", q, k) / np.sqrt(np.float32(DK))
    sc = np.where(mask == 0, np.float32(-1e9), sc)
    sc = sc - sc.max(axis=-1, keepdims=True)
    e = np.exp(sc)
    attn = e / e.sum(axis=-1, keepdims=True)
    cx = np.einsum("bhqk,bhkd->bhqd", attn, v)
    cx = cx.transpose(0, 2, 1, 3).reshape(B, S, D)
    return lora(cx, Wo, Ao, Bo).astype(np.float32)


def _prepare_in_maps(query, key, value, Wq, Aq, Bq, Wk, Ak, Bk, Wv, Av, Bv, Wo, Ao, Bo):
    f32 = np.float32
    bf16 = ml_dtypes.bfloat16
    weff = {}
    for n, (W, A, Bm) in {
        "q": (Wq, Aq, Bq),
        "k": (Wk, Ak, Bk),
        "v": (Wv, Av, Bv),
        "o": (Wo, Ao, Bo),
    }.items():
        weff[n] = (
            np.asarray(W, f32) + SCALING * np.asarray(Bm, f32) @ np.asarray(A, f32)
        ).astype(f32)

    xT = {
        "q": [np.ascontiguousarray(np.asarray(query[b], f32).T).astype(bf16) for b in range(B)],
        "k": [np.ascontiguousarray(np.asarray(key[b], f32).T).astype(bf16) for b in range(B)],
        "v": [np.ascontiguousarray(np.asarray(value[b], f32).T).astype(bf16) for b in range(B)],
    }
    in_maps = []
    for c in range(N_CORES):
        b, g = divmod(c, CPG)
        cs = slice(CSLICE * g, CSLICE * (g + 1))
        in_maps.append(
            {
                "xqT": xT["q"][b],
                "xkT": xT["k"][b],
                "xvT": xT["v"][b],
                "wq": np.ascontiguousarray(weff["q"][cs, :].T).astype(bf16),
                "wk": np.ascontiguousarray(weff["k"][cs, :].T).astype(bf16),
                "wv": np.ascontiguousarray(weff["v"][cs, :].T).astype(bf16),
                "wo": np.ascontiguousarray(weff["o"][:, cs].T).astype(bf16),
            }
        )
    return in_maps


def run(inputs, trace=False, **spmd_kwargs):
    """Shard, run on 8 cores, gather.  Returns (output, BassKernelResults)."""
    mask = np.asarray(inputs["mask"])
    if not np.all(mask != 0):
        out = _numpy_reference(
            np.asarray(inputs["query"], np.float32),
            np.asarray(inputs["key"], np.float32),
            np.asarray(inputs["value"], np.float32),
            mask,
            *[
                np.asarray(inputs[k], np.float32)
                for k in ("Wq", "Aq", "Bq", "Wk", "Ak", "Bk", "Wv", "Av", "Bv", "Wo", "Ao", "Bo")
            ],
        )
        return out, None

    in_maps = _prepare_in_maps(
        inputs["query"], inputs["key"], inputs["value"],
        inputs["Wq"], inputs["Aq"], inputs["Bq"],
        inputs["Wk"], inputs["Ak"], inputs["Bk"],
        inputs["Wv"], inputs["Av"], inputs["Bv"],
        inputs["Wo"], inputs["Ao"], inputs["Bo"],
    )
    nc = _get_nc()
    res = run_bass_kernel_spmd(
        nc, in_maps, core_ids=list(range(N_CORES)), trace=trace, **spmd_kwargs
    )
    out = np.empty((B, S, D), np.float32)
    for b in range(B):
        acc = res.results[CPG * b]["outT"].astype(np.float32)
        for g in range(1, CPG):
            acc = acc + res.results[CPG * b + g]["outT"].astype(np.float32)
        out[b] = acc.T
    return out, res


def kernel(**inputs):
    out, _ = run(inputs, trace=False)
    return out



# revision 25
# speedup vs baseline: 1.1642x; 1.1642x over previous
"""LoRA multi-head attention on 8 Trainium2 NeuronCores.

Sharding: data-parallel over batch (B=2) x tensor-parallel over heads
(16 heads -> 4 per core).  Core c handles batch b=c//4 and head group
g=c%4 (columns C=[256*g, 256*g+256) of the projection output).

Host prep (per weight): W_eff = W + 2.0 * B @ A  (exact LoRA fold),
transposed activations x.T, everything bf16.  Output partials ship
bf16 and are summed on host across the 4 cores of each batch.

Device schedule per core:
  DMA: 256KB half-row transfers; sync queue carries wk+xk, the Pool
  queue carries wq/xq/wv/xv/wo.  Nothing rides the ACT queue (exps
  must not sit behind DMAs).  Prefix: K st0/st1 + Q0 projections as
  one i-loop ordered by DMA arrival, accumulating in the idle sc/pj
  PSUM banks (2KB zero regions keep interleaved groups independent).
  K st2/st3, the V projection (split by head-pair), and Q proj of
  qt1..3 are woven into the attention groups as per-step fillers.

  8 attention groups (q-tile x head-pair), 16 t-steps each: scores
  pair (PE, row-tiled co-issue) -> exp -> PV pair + softmax row-sums
  via a ones-column in V.  exp is split 10/6 between the ACT engine
  (table Exp) and a custom 8-stage DVE op exp(y)~=(1+y/64)^64 (the
  softmax tolerates this: numerator and denominator shift together).
  PV emission lags exp by 2+ steps so the PE never waits.  The output
  projection of qt-1 rides 2 groups later; qt3's final o-blocks open
  their first contraction half inside g7.  PSUM evictions split
  across ACT/DVE (the Pool engine cannot touch PSUM and reprograms
  between op types, so it only does partition_broadcast + DMA); ctx
  normalization: DVE reciprocal of the row-sum row, Pool broadcast,
  DVE multiply reading ctx PSUM directly.

PSUM (8 banks): sc 2x[128,1024]=4, pj 2x[128,512]=2, ctx 2x[65,512]=2.
"""

import sys

sys.path.insert(0, "/opt/trn_rl_repo")

from contextlib import ExitStack

import ml_dtypes
import numpy as np

import concourse.bass as bass
import concourse.tile as tile
from concourse import bacc, mybir
from concourse.bass_utils import run_bass_kernel_spmd

# ---- custom DVE exp op ----------------------------------------------------
import concourse.dve_ops as dve_ops
from concourse.dve_spec import C0, C1, Spec, Src0, sq
from concourse.dve_spec import lower as dve_lower
from concourse.dve_uop import DveOpSpec


def _exp64_ref(in0, in1, s0, s1, imm2):
    z = (in0.astype(np.float32) * np.float32(s0) + np.float32(s1)).astype(np.float32)
    for _ in range(6):
        z = z * z
    return z


def _register_exp_op():
    for op in dve_ops.OPS:
        if op.name == "EXP64_ANT":
            return op
    body = Src0 * C0 + C1
    for _ in range(6):
        body = sq(body)
    spec = Spec(body=body, reference=_exp64_ref)
    row = max(dve_ops._SUB_OPCODE_FOR_NAME.values()) + 1
    shas = {}
    for ver in ("v3", "v4"):
        uops = dve_lower(spec, ver=ver)
        shas[ver] = DveOpSpec(
            name="EXP64_ANT", opcode=row, uops=uops, rd1_en=False
        ).sha(ver)
    op = dve_ops.DveOp("EXP64_ANT", spec, subdim=False, uops_sha=shas)
    dve_ops.OPS.append(op)
    dve_ops._SUB_OPCODE_FOR_NAME[op.name] = row
    dve_ops.CUSTOM_DVE_SPECS[op.name] = spec
    return op


EXP_OP = _register_exp_op()

F32 = mybir.dt.float32
BF16 = mybir.dt.bfloat16

B = 2
S = 2048
D = 1024
H = 16
DK = 64
SCALING = 2.0
N_CORES = 8
CPG = 4
CSLICE = D // CPG
Exp = mybir.ActivationFunctionType.Exp
MULT = mybir.AluOpType.mult

# exp-engine routing: which t-steps run on the DVE (rest on ACT)
DVE_STEPS = frozenset({1, 4, 7, 9, 12, 14})
DVE_STEPS_G0 = frozenset({5, 8, 10, 12, 14})

_CACHE = {}


def _build():
    nc = bacc.Bacc("TRN2", target_bir_lowering=False, debug=False)

    xqT = nc.declare_dram_parameter("xqT", [D, S], BF16, isOutput=False)
    xkT = nc.declare_dram_parameter("xkT", [D, S], BF16, isOutput=False)
    xvT = nc.declare_dram_parameter("xvT", [D, S], BF16, isOutput=False)
    wq = nc.declare_dram_parameter("wq", [D, CSLICE], BF16, isOutput=False)
    wk = nc.declare_dram_parameter("wk", [D, CSLICE], BF16, isOutput=False)
    wv = nc.declare_dram_parameter("wv", [D, CSLICE], BF16, isOutput=False)
    wo = nc.declare_dram_parameter("wo", [CSLICE, D], BF16, isOutput=False)
    wo3 = nc.declare_dram_parameter("wo3", [DK, D], BF16, isOutput=False)
    outT = nc.declare_dram_parameter("outT", [D, S], BF16, isOutput=True)

    with tile.TileContext(nc) as tc, ExitStack() as ctx:
        const = ctx.enter_context(tc.tile_pool(name="const", bufs=1))
        xkp = ctx.enter_context(tc.tile_pool(name="xkp", bufs=16))
        xvp = ctx.enter_context(tc.tile_pool(name="xvp", bufs=16))
        xqp = ctx.enter_context(tc.tile_pool(name="xqp", bufs=8))
        expp = ctx.enter_context(tc.tile_pool(name="expp", bufs=16))
        smallp = ctx.enter_context(tc.tile_pool(name="smallp", bufs=2))
        psum = ctx.enter_context(tc.tile_pool(name="psum", bufs=2, space="PSUM"))

        wq_sb = const.tile([128, 8, CSLICE], BF16)
        wk_sb = const.tile([128, 8, CSLICE], BF16)
        wv_sb = const.tile([128, 8, CSLICE], BF16)
        wo_sb = const.tile([128, 2, D], BF16)
        wo3_sb = const.tile([DK, D], BF16)

        kT_s = [const.tile([128, 2, 512], BF16, name=f"kT{i}") for i in range(4)]
        qT_s = [const.tile([128, 2, 512], BF16, name=f"qT{i}") for i in range(4)]
        # v[tt]: [128 kpos, t4, head, dk+ones]
        v_s = [const.tile([128, 4, 4, DK + 1], BF16, name=f"v_{i}") for i in range(4)]
        ctxT_s = [const.tile([128, 2, 512], BF16, name=f"cx{i}") for i in range(4)]

        # ---- input DMAs: 256KB half-row transfers, none on the scalar
        # queue (ACT must not queue exps behind DMAs).  Left halves first
        # so K-st0/st1 and Q0 can start ~8us in.
        xkL, xkR, xvL, xvR, xq_b, xq23_b = [], [], [], [], [], []
        nc.sync.dma_start(wk_sb[:], wk.rearrange("(i p) c -> p i c", p=128))
        for half, lst in ((0, xkL), (1, xkR)):
            for i in range(8):
                t_ = xkp.tile([128, 1024], BF16, tag="xk", bufs=16, name=f"xk{half}_{i}")
                nc.sync.dma_start(
                    t_[:], xkT[128 * i : 128 * (i + 1), 1024 * half : 1024 * (half + 1)]
                )
                lst.append(t_)
        nc.gpsimd.dma_start(wq_sb[:], wq.rearrange("(i p) c -> p i c", p=128))
        for i in range(8):
            t_ = xqp.tile([128, 1024], BF16, tag="xq", bufs=8, name=f"xq01_{i}")
            nc.gpsimd.dma_start(t_[:], xqT[128 * i : 128 * (i + 1), 0:1024])
            xq_b.append(t_)
        nc.gpsimd.dma_start(wv_sb[:], wv.rearrange("(i p) c -> p i c", p=128))
        for half, lst in ((0, xvL), (1, xvR)):
            for i in range(8):
                t_ = xvp.tile([128, 1024], BF16, tag="xv", bufs=16, name=f"xv{half}_{i}")
                nc.gpsimd.dma_start(
                    t_[:], xvT[128 * i : 128 * (i + 1), 1024 * half : 1024 * (half + 1)]
                )
                lst.append(t_)
        nc.gpsimd.dma_start(wo_sb[:], wo.rearrange("(c p) o -> p c o", p=128))
        nc.gpsimd.dma_start(wo3_sb[:], wo3[:, :])

        # ones column of V (softmax row sums ride the PV matmul)
        for tt in range(4):
            nc.vector.memset(v_s[tt][:, :, :, DK : DK + 1], 1.0)

        # ---- prefix: K st0/st1 + Q0, i-loop ordered by DMA arrival ----
        # Accumulators: two sc-tag PSUM tiles hold st0/st1 x (cc0|cc1) as
        # 2KB halves (zero regions are 2KB so the interleaved groups are
        # independent); Q0 rides the two pj slots.  One [128,1024] DVE
        # eviction per kT tile.  K st2/st3 and V ride inside g0 as
        # fillers; Q1 inside g1.
        sH = [
            [
                psum.tile([128, 512], F32, tag="sc", bufs=4, name=f"kl{st}{cc}")
                for cc in range(2)
            ]
            for st in range(2)
        ]
        pjQ = [
            psum.tile([128, 512], F32, tag="pj", bufs=2, name=f"q0_{cc}")
            for cc in range(2)
        ]
        for i in range(8):
            st_, sp_ = (i == 0), (i == 7)
            for cc in range(2):
                wsl = wk_sb[:, i, 128 * cc : 128 * (cc + 1)]
                nc.tensor.matmul(
                    sH[0][cc][:], wsl, xkL[i][:, 0:512],
                    start=st_, stop=sp_,
                )
                nc.tensor.matmul(
                    sH[1][cc][:], wsl, xkL[i][:, 512:1024],
                    start=st_, stop=sp_,
                )
            for cc in range(2):
                nc.tensor.matmul(
                    pjQ[cc][:],
                    wq_sb[:, i, 128 * cc : 128 * (cc + 1)],
                    xq_b[i][:, 0:512],
                    start=st_, stop=sp_,
                )
        nc.vector.tensor_copy(kT_s[0][:, 0, :], sH[0][0][:])
        nc.vector.tensor_copy(kT_s[0][:, 1, :], sH[0][1][:])
        nc.scalar.copy(kT_s[1][:, 0, :], sH[1][0][:])
        nc.scalar.copy(kT_s[1][:, 1, :], sH[1][1][:])
        nc.vector.tensor_copy(qT_s[0][:, 0, :], pjQ[0][:])
        nc.scalar.copy(qT_s[0][:, 1, :], pjQ[1][:])

        kr_live = {}

        def kr_mm(st, cc, i):
            # K st2/st3 round, woven into g0 (4 mm/step)
            if i == 0:
                kr_live[(st, cc)] = psum.tile(
                    [128, 512], F32, tag="pj", bufs=2, name=f"kr{st}{cc}"
                )
            ps = kr_live[(st, cc)]
            nc.tensor.matmul(
                ps[:],
                wk_sb[:, i, 128 * cc : 128 * (cc + 1)],
                (xkR[i][:, 0:512] if st == 2 else xkR[i][:, 512:1024]),
                start=(i == 0),
                stop=(i == 7),
            )
            if i == 7:
                ps = kr_live.pop((st, cc))
                if cc == 0:
                    nc.scalar.copy(kT_s[st][:, 0, :], ps[:])
                else:
                    nc.vector.tensor_copy(kT_s[st][:, 1, :], ps[:])

        # ---- attention building blocks --------------------------------
        def scores_pair(qt, p, t):
            scE = psum.tile([128, 512], F32, tag="sc", bufs=4, name=f"sE{qt}{p}{t}")
            scO = psum.tile([128, 512], F32, tag="sc", bufs=4, name=f"sO{qt}{p}{t}")
            kt = kT_s[t // 4]
            ts_ = slice(128 * (t % 4), 128 * (t % 4 + 1))
            qtile = qT_s[qt]
            nc.tensor.matmul(
                scE[:],
                kt[0:64, p, ts_],
                qtile[0:64, p, :],
                start=True,
                stop=True,
                tile_position=(0, 0),
            )
            nc.tensor.matmul(
                scO[:],
                kt[64:128, p, ts_],
                qtile[64:128, p, :],
                start=True,
                stop=True,
                tile_position=(64, 0),
            )
            return scE, scO

        def exp_emit(qt, p, t, scE, scO):
            etE = expp.tile([128, 512], BF16, tag="et", bufs=32, name=f"eE{qt}{p}{t}")
            etO = expp.tile([128, 512], BF16, tag="et", bufs=32, name=f"eO{qt}{p}{t}")
            nc.scalar.activation(etE[:], scE[:], Exp, scale=1.0 / 8.0)
            if t in O_ACT_STEPS:
                nc.scalar.activation(etO[:], scO[:], Exp, scale=1.0 / 8.0)
            else:
                nc.vector._custom_dve(
                    EXP_OP, out=etO[:], in0=scO[:], s0=1.0 / 512.0, s1=1.0
                )
            return etE, etO

        def pv_emit(p, t, et, ctx0, ctx1):
            etE, etO = et
            vg = v_s[t // 4]
            nc.tensor.matmul(
                ctx0[:],
                vg[:, t % 4, 2 * p, :],
                etE[:],
                start=(t == 0),
                stop=(t == 15),
            )
            nc.tensor.matmul(
                ctx1[:],
                vg[:, t % 4, 2 * p + 1, :],
                etO[:],
                start=(t == 0),
                stop=(t == 15),
            )

        def v_round(hp, tt, t4):
            ps = psum.tile([128, 128], F32, tag="pj", bufs=2, name=f"vps{hp}{tt}{t4}")
            xv_half = xvL if tt < 2 else xvR
            c0 = 512 * (tt % 2) + 128 * t4
            for i in range(8):
                nc.tensor.matmul(
                    ps[:],
                    xv_half[i][:, c0 : c0 + 128],
                    wv_sb[:, i, 128 * hp : 128 * (hp + 1)],
                    start=(i == 0),
                    stop=(i == 7),
                )
            dst = v_s[tt][:, t4, 2 * hp : 2 * hp + 2, 0:DK]
            srcv = ps[:].rearrange("p (h d) -> p h d", h=2)
            if t4 % 2 == 0:
                nc.scalar.copy(dst, srcv)
            else:
                nc.vector.tensor_copy(dst, srcv)

        def oblock(qt, o):
            ops = psum.tile([128, 512], F32, tag="pj", bufs=2, name=f"op{qt}_{o}")
            nc.tensor.matmul(
                ops[:],
                wo_sb[:, 0, 128 * o : 128 * (o + 1)],
                ctxT_s[qt][:, 0, :],
                start=True,
                stop=False,
            )
            nc.tensor.matmul(
                ops[:],
                wo_sb[:, 1, 128 * o : 128 * (o + 1)],
                ctxT_s[qt][:, 1, :],
                start=False,
                stop=True,
            )
            ob = smallp.tile([128, 512], BF16, tag="ob", bufs=3)
            if o % 2 == 0:
                nc.scalar.copy(ob[:], ops[:])
            else:
                nc.vector.tensor_copy(ob[:], ops[:])
            nc.sync.dma_start(
                outT[128 * o : 128 * (o + 1), 512 * qt : 512 * (qt + 1)], ob[:]
            )

        ob3_held = {}

        def ob3_c0_open(o):
            # first contraction half, opened during the tail norm window
            ops = psum.tile([128, 512], F32, tag="pj", bufs=2, name=f"o3h{o}")
            nc.tensor.matmul(
                ops[:],
                wo_sb[:, 0, 128 * o : 128 * (o + 1)],
                ctxT_s[3][:, 0, :],
                start=True,
                stop=False,
            )
            ob3_held[o] = ops

        def ob3_block(o):
            # heads 0/1 via K=128 c0; head 2 via K=64 on ctxT even half;
            # head 3 via K=64 on the ct tile through wo3 (no ctxT DMA)
            ops = ob3_held.pop(o, None)
            if ops is None:
                ops = psum.tile([128, 512], F32, tag="pj", bufs=2, name=f"o3{o}")
                nc.tensor.matmul(
                    ops[:],
                    wo_sb[:, 0, 128 * o : 128 * (o + 1)],
                    ctxT_s[3][:, 0, :],
                    start=True,
                    stop=False,
                )
            nc.tensor.matmul(
                ops[:],
                wo_sb[0:DK, 1, 128 * o : 128 * (o + 1)],
                ctxT_s[3][0:DK, 1, :],
                start=False,
                stop=False,
            )
            nc.tensor.matmul(
                ops[:],
                wo3_sb[:, 128 * o : 128 * (o + 1)],
                ct_store["ct3"][:],
                start=False,
                stop=True,
            )
            ob = smallp.tile([128, 512], BF16, tag="ob", bufs=3)
            if o % 2 == 0:
                nc.scalar.copy(ob[:], ops[:])
            else:
                nc.vector.tensor_copy(ob[:], ops[:])
            nc.sync.dma_start(
                outT[128 * o : 128 * (o + 1), 512 * 3 : 512 * 4], ob[:]
            )

        # Q proj for qt 1/2/3, spread across steps (pj slot held across)
        qproj_live = {}

        def qproj_mm(qtn, cc, i):
            if i == 0:
                qproj_live[(qtn, cc)] = psum.tile(
                    [128, 512], F32, tag="pj", bufs=2, name=f"qp{qtn}{cc}"
                )
            ps = qproj_live[(qtn, cc)]
            xsrc = (
                xq_b[i][:, 512:1024]
                if qtn == 1
                else xq23_b[i][:, 512 * (qtn - 2) : 512 * (qtn - 1)]
            )
            nc.tensor.matmul(
                ps[:],
                wq_sb[:, i, 128 * cc : 128 * (cc + 1)],
                xsrc,
                start=(i == 0),
                stop=(i == 7),
            )
            if i == 7:
                nc.scalar.copy(qT_s[qtn][:, cc, :], qproj_live.pop((qtn, cc)))

        def xq_load(qtn):
            if qtn != 2:
                return
            for i in range(8):
                t_ = xqp.tile([128, 1024], BF16, tag="xq", bufs=8, name=f"xq23_{i}")
                nc.sync.dma_start(t_[:], xqT[128 * i : 128 * (i + 1), 1024:2048])
                xq23_b.append(t_)

        def mk_norm(cxE, cxO, qt, p):
            # both rowsums sit at psum row 64; stage them to partition 0
            # (proven cross-partition [1,512] copies), one fused reciprocal
            rs = smallp.tile([1, 2, 512], F32, tag="rs1", bufs=2)
            nc.vector.tensor_copy(rs[:, 0, :], cxE[DK : DK + 1, :])
            nc.vector.tensor_copy(rs[:, 1, :], cxO[DK : DK + 1, :])
            rc = smallp.tile([1, 2, 512], F32, tag="rc", bufs=2)
            nc.vector.reciprocal_approx_fast(rc[:], rs[:])
            bcE = smallp.tile([64, 512], F32, tag="bcE", bufs=2)
            nc.gpsimd.partition_broadcast(bcE[:], rc[:, 0, :])
            bcO = smallp.tile([64, 512], F32, tag="bcO", bufs=2)
            nc.gpsimd.partition_broadcast(bcO[:], rc[:, 1, :])
            nc.vector.tensor_tensor(
                ctxT_s[qt][0:DK, p, :], cxE[0:DK, :], bcE[:], MULT
            )
            ct = smallp.tile([64, 512], BF16, tag="ct", bufs=2)
            nc.vector.tensor_tensor(ct[:], cxO[0:DK, :], bcO[:], MULT)
            nc.sync.dma_start(ctxT_s[qt][DK : 2 * DK, p, :], ct[:])

        # ---- attention groups -----------------------------------------
        # pending: work carried into the next group's first steps
        pending = []

        def drain_pending(upto):
            while pending and pending[0][0] <= upto:
                pending.pop(0)[1]()

        def emit_group(qt, p):
            gi = 2 * qt + p
            ctx0 = psum.tile([DK + 1, 512], F32, tag="ctxE", bufs=1, name=f"cx{qt}{p}0")
            ctx1 = psum.tile([DK + 1, 512], F32, tag="ctxO", bufs=1, name=f"cx{qt}{p}1")
            ets = {}

            # per-step pv emission plan (lagged so the PE never waits on exp,
            # and so every v_round/kr_mm a pv needs precedes it in the queue)
            if gi == 0:
                pv_plan = {10: [0, 1], 11: [2, 3], 12: [4, 5], 13: [6, 7, 8],
                           14: [9, 10, 11], 15: [12, 13]}
            elif gi == 1:
                pv_plan = {8: [0, 1], 9: [2, 3], 10: [4, 5], 11: [6, 7],
                           12: [8, 9], 13: [10, 11], 14: [12, 13]}
            else:
                pv_plan = {t: [t - 2] for t in range(2, 16)}

            # per-step filler plan
            fillers = {t: [] for t in range(16)}
            if gi == 0:
                # K st2/st3 (4 mm/step, steps 0-7), V heads 0/1 (2/step, 8-15)
                for j in range(32):
                    st, cc, i = 2 + j // 16, (j % 16) // 8, j % 8
                    fillers[2 + j // 4].append(lambda st=st, cc=cc, i=i: kr_mm(st, cc, i))
                for j in range(16):
                    fillers[8 + j // 2].append(
                        lambda j=j: v_round(0, j // 4, j % 4)
                    )
            elif gi == 1:
                # V heads 2/3 (2/step, steps 0-7), Q1 (2 mm/step, 8-15)
                for j in range(16):
                    fillers[j // 2].append(lambda j=j: v_round(1, j // 4, j % 4))
                for j in range(16):
                    cc, i = j // 8, j % 8
                    fillers[8 + j // 2].append(lambda cc=cc, i=i: qproj_mm(1, cc, i))
            else:
                if gi in (2, 4):
                    fillers[2].append(lambda qtn=qt + 1: xq_load(qtn))
                if gi in (2, 3, 4, 5):
                    qtn, cc = qt + 1, p
                    i = 0
                    for t_, n_ in ((10, 1), (11, 1), (12, 1), (13, 1), (14, 2), (15, 2)):
                        for _ in range(n_):
                            fillers[t_].append(
                                lambda qtn=qtn, cc=cc, i=i: qproj_mm(qtn, cc, i)
                            )
                            i += 1
                ob_base = 4 * p
                for j, t in enumerate((3, 5, 7, 9)):
                    fillers[t].append(lambda qt=qt, o=ob_base + j: oblock(qt - 1, o))
                if gi == 7:
                    for j, t in enumerate((10, 12, 13, 15)):
                        fillers[t].append(lambda o=j: ob3_partial_c0(o))

            for t in range(16):
                if t == 0:
                    drain_pending(0)
                sc = scores_pair(qt, p, t)
                ets[t] = exp_emit(qt, p, t, sc)
                if t == 1:
                    drain_pending(1)
                for tp in pv_plan.get(t, ()):
                    pv_emit(p, tp, ets.pop(tp), ctx0, ctx1)
                for f in fillers[t]:
                    f()

            # carry the drain into the next group
            def fin(qt=qt, p=p, ctx0=ctx0, ctx1=ctx1, ets=ets):
                pv_emit(p, 14, ets.pop(14), ctx0, ctx1)
                pv_emit(p, 15, ets.pop(15), ctx0, ctx1)
                mk_norm(ctx0, ctx1, qt, p)

            if qt == 3 and p == 1:
                fin()
            else:
                pending.append((0, fin))

        for qt in range(4):
            for p in range(2):
                emit_group(qt, p)

        # ---- tail: qt3 output projection.  c0 of blocks 0/1 opens during
        # the norm-chain window; every block then adds head 2 from the even
        # ctxT half and head 3 straight from the ct tile via wo3, so nothing
        # waits on a ctxT DMA.
        ob3_c0_open(0)
        ob3_c0_open(1)
        for o in range(8):
            ob3_block(o)

    nc.finalize()
    return nc


def _get_nc():
    if "nc" not in _CACHE:
        _CACHE["nc"] = _build()
    return _CACHE["nc"]


def _numpy_reference(query, key, value, mask, Wq, Aq, Bq, Wk, Ak, Bk, Wv, Av, Bv, Wo, Ao, Bo):
    """Exact fallback for a non-all-ones mask (never hit for the spec'd inputs)."""

    def lora(x, W, A, Bm):
        return x @ W.T + ((x @ A.T) @ Bm.T) * SCALING

    q = lora(query, Wq, Aq, Bq).reshape(B, S, H, DK).transpose(0, 2, 1, 3)
    k = lora(key, Wk, Ak, Bk).reshape(B, S, H, DK).transpose(0, 2, 1, 3)
    v = lora(value, Wv, Av, Bv).reshape(B, S, H, DK).transpose(0, 2, 1, 3)
    sc = np.einsum("bhqd,bhkd->bhqk", q, k) / np.sqrt(np.float32(DK))
    sc = np.where(mask == 0, np.float32(-1e9), sc)
    sc = sc - sc.max(axis=-1, keepdims=True)
    e = np.exp(sc)
    attn = e / e.sum(axis=-1, keepdims=True)
    cx = np.einsum("bhqk,bhkd->bhqd", attn, v)
    cx = cx.transpose(0, 2, 1, 3).reshape(B, S, D)
    return lora(cx, Wo, Ao, Bo).astype(np.float32)


def _prepare_in_maps(query, key, value, Wq, Aq, Bq, Wk, Ak, Bk, Wv, Av, Bv, Wo, Ao, Bo):
    f32 = np.float32
    bf16 = ml_dtypes.bfloat16
    weff = {}
    for n, (W, A, Bm) in {
        "q": (Wq, Aq, Bq),
        "k": (Wk, Ak, Bk),
        "v": (Wv, Av, Bv),
        "o": (Wo, Ao, Bo),
    }.items():
        weff[n] = (
            np.asarray(W, f32) + SCALING * np.asarray(Bm, f32) @ np.asarray(A, f32)
        ).astype(f32)

    xT = {
        "q": [np.ascontiguousarray(np.asarray(query[b], f32).T).astype(bf16) for b in range(B)],
        "k": [np.ascontiguousarray(np.asarray(key[b], f32).T).astype(bf16) for b in range(B)],
        "v": [np.ascontiguousarray(np.asarray(value[b], f32).T).astype(bf16) for b in range(B)],
    }
    in_maps = []
    for c in range(N_CORES):
        b, g = divmod(c, CPG)
        cs = slice(CSLICE * g, CSLICE * (g + 1))
        wo_arr = np.ascontiguousarray(weff["o"][:, cs].T).astype(bf16)
        in_maps.append(
            {
                "xqT": xT["q"][b],
                "xkT": xT["k"][b],
                "xvT": xT["v"][b],
                "wq": np.ascontiguousarray(weff["q"][cs, :].T).astype(bf16),
                "wk": np.ascontiguousarray(weff["k"][cs, :].T).astype(bf16),
                "wv": np.ascontiguousarray(weff["v"][cs, :].T).astype(bf16),
                "wo": wo_arr,
                "wo3": np.ascontiguousarray(wo_arr[192:256, :]),
            }
        )
    return in_maps


def run(inputs, trace=False, **spmd_kwargs):
    """Shard, run on 8 cores, gather.  Returns (output, BassKernelResults)."""
    mask = np.asarray(inputs["mask"])
    if not np.all(mask != 0):
        out = _numpy_reference(
            np.asarray(inputs["query"], np.float32),
            np.asarray(inputs["key"], np.float32),
            np.asarray(inputs["value"], np.float32),
            mask,
            *[
                np.asarray(inputs[k], np.float32)
                for k in ("Wq", "Aq", "Bq", "Wk", "Ak", "Bk", "Wv", "Av", "Bv", "Wo", "Ao", "Bo")
            ],
        )
        return out, None

    in_maps = _prepare_in_maps(
        inputs["query"], inputs["key"], inputs["value"],
        inputs["Wq"], inputs["Aq"], inputs["Bq"],
        inputs["Wk"], inputs["Ak"], inputs["Bk"],
        inputs["Wv"], inputs["Av"], inputs["Bv"],
        inputs["Wo"], inputs["Ao"], inputs["Bo"],
    )
    nc = _get_nc()
    res = run_bass_kernel_spmd(
        nc, in_maps, core_ids=list(range(N_CORES)), trace=trace, **spmd_kwargs
    )
    out = np.empty((B, S, D), np.float32)
    for b in range(B):
        acc = res.results[CPG * b]["outT"].astype(np.float32)
        for g in range(1, CPG):
            acc = acc + res.results[CPG * b + g]["outT"].astype(np.float32)
        out[b] = acc.T
    return out, res


def kernel(**inputs):
    out, _ = run(inputs, trace=False)
    return out



# revision 27
# speedup vs baseline: 1.1712x; 1.0060x over previous
"""LoRA multi-head attention on 8 Trainium2 NeuronCores.

Sharding: data-parallel over batch (B=2) x tensor-parallel over heads
(16 heads -> 4 per core).  Core c handles batch b=c//4 and head group
g=c%4 (columns C=[256*g, 256*g+256) of the projection output).

Host prep (per weight): W_eff = W + 2.0 * B @ A  (exact LoRA fold),
transposed activations x.T, everything bf16.  Output partials ship
bf16 and are summed on host across the 4 cores of each batch.

Device schedule per core:
  DMA: 256KB half-row transfers (2KB partition lines; narrower tiles
  halve DMA throughput).  sync queue: wk, xkL, xkR, xvR; Pool queue:
  wq, xq01, wv, xvL, wo, wo3.  Nothing rides the ACT queue (exps must
  not sit behind DMAs).  Prefix: K st0/st1 + Q0 projections as one
  i-loop ordered by DMA arrival.  K st2/st3, the V projection (one
  N=256 chain per (tt,t4) block computing all 4 heads -- per-pair
  N=128 rounds are LDWEIGHTS-bound), and Q proj of qt1..3 are woven
  into the attention groups as fillers.

  8 attention groups (q-tile x head-pair), 8 two-step batches each.
  Scores/exp run at half-tile [128,512] granularity: the sc tag ring
  is 4 half-banks, and each step's two half-exps run CONCURRENTLY on
  ACT (table Exp) and DVE (custom 8-stage op exp(y)~=(1+y/64)^64; the
  softmax tolerates it: numerator and denominator shift together).
  This keeps the sc-bank-reuse chain (scores(t) needs exp(t-2)) off
  the critical path -- with full-tile exps its ~1.15us latency paced
  every step.  Each group emits its own pvs for steps 0-9 in batches
  b4-b7 and spills steps 10-15 + the norm chain into the next group's
  first batches, which otherwise run bare scores while the exp chain
  catches up.  The output projection of qt-1 rides 2 groups later.
  ctx normalization: cross-partition [1,512] copies of the rowsum
  rows to partition 0, reciprocal, Pool broadcasts (E first), DVE
  multiplies (odd half -> ct tile -> DMA to ctxT partitions 64:128;
  the Pool engine cannot touch PSUM, and reciprocal_approx_fast
  reading PSUM directly gives wrong results on HW).

  Tail: qt3's o-blocks open their first contraction half early into
  the pj banks plus four sc half-banks (free after the last exps);
  after the norm chain each block adds head 2 from ctxT's even half
  and head 3 straight from the ct tile via wo3 (head-3 rows staged at
  partitions 0:64, K=64 matmul) so nothing waits on a ctxT DMA.

PSUM (8 banks): sc 4x[128,512]=4, pj 2x[128,512]=2,
ctxE [65,512] + ctxO [65,512] = 2.
"""

import sys

sys.path.insert(0, "/opt/trn_rl_repo")

from contextlib import ExitStack

import ml_dtypes
import numpy as np

import concourse.bass as bass
import concourse.tile as tile
from concourse import bacc, mybir
from concourse.bass_utils import run_bass_kernel_spmd

# ---- custom DVE exp op ----------------------------------------------------
import concourse.dve_ops as dve_ops
from concourse.dve_spec import C0, C1, Spec, Src0, sq
from concourse.dve_spec import lower as dve_lower
from concourse.dve_uop import DveOpSpec


def _exp64_ref(in0, in1, s0, s1, imm2):
    z = (in0.astype(np.float32) * np.float32(s0) + np.float32(s1)).astype(np.float32)
    for _ in range(6):
        z = z * z
    return z


def _register_exp_op():
    for op in dve_ops.OPS:
        if op.name == "EXP64_ANT":
            return op
    body = Src0 * C0 + C1
    for _ in range(6):
        body = sq(body)
    spec = Spec(body=body, reference=_exp64_ref)
    row = max(dve_ops._SUB_OPCODE_FOR_NAME.values()) + 1
    shas = {}
    for ver in ("v3", "v4"):
        uops = dve_lower(spec, ver=ver)
        shas[ver] = DveOpSpec(
            name="EXP64_ANT", opcode=row, uops=uops, rd1_en=False
        ).sha(ver)
    op = dve_ops.DveOp("EXP64_ANT", spec, subdim=False, uops_sha=shas)
    dve_ops.OPS.append(op)
    dve_ops._SUB_OPCODE_FOR_NAME[op.name] = row
    dve_ops.CUSTOM_DVE_SPECS[op.name] = spec
    return op


EXP_OP = _register_exp_op()

F32 = mybir.dt.float32
BF16 = mybir.dt.bfloat16

B = 2
S = 2048
D = 1024
H = 16
DK = 64
SCALING = 2.0
N_CORES = 8
CPG = 4
CSLICE = D // CPG
Exp = mybir.ActivationFunctionType.Exp
MULT = mybir.AluOpType.mult

# exp-engine routing: which t-steps run on the DVE (rest on ACT)
DVE_STEPS = frozenset({1, 4, 7, 9, 12, 14})
DVE_STEPS_G0 = frozenset({5, 8, 10, 12, 14})

_CACHE = {}


def _build():
    nc = bacc.Bacc("TRN2", target_bir_lowering=False, debug=False)

    xqT = nc.declare_dram_parameter("xqT", [D, S], BF16, isOutput=False)
    xkT = nc.declare_dram_parameter("xkT", [D, S], BF16, isOutput=False)
    xvT = nc.declare_dram_parameter("xvT", [D, S], BF16, isOutput=False)
    wq = nc.declare_dram_parameter("wq", [D, CSLICE], BF16, isOutput=False)
    wk = nc.declare_dram_parameter("wk", [D, CSLICE], BF16, isOutput=False)
    wv = nc.declare_dram_parameter("wv", [D, CSLICE], BF16, isOutput=False)
    wo = nc.declare_dram_parameter("wo", [CSLICE, D], BF16, isOutput=False)
    wo3 = nc.declare_dram_parameter("wo3", [DK, D], BF16, isOutput=False)
    outT = nc.declare_dram_parameter("outT", [D, S], BF16, isOutput=True)

    with tile.TileContext(nc) as tc, ExitStack() as ctx:
        const = ctx.enter_context(tc.tile_pool(name="const", bufs=1))
        xkp = ctx.enter_context(tc.tile_pool(name="xkp", bufs=16))
        xvp = ctx.enter_context(tc.tile_pool(name="xvp", bufs=16))
        xqp = ctx.enter_context(tc.tile_pool(name="xqp", bufs=8))
        expp = ctx.enter_context(tc.tile_pool(name="expp", bufs=16))
        smallp = ctx.enter_context(tc.tile_pool(name="smallp", bufs=2))
        psum = ctx.enter_context(tc.tile_pool(name="psum", bufs=2, space="PSUM"))

        wq_sb = const.tile([128, 8, CSLICE], BF16)
        wk_sb = const.tile([128, 8, CSLICE], BF16)
        wv_sb = const.tile([128, 8, CSLICE], BF16)
        wo_sb = const.tile([128, 2, D], BF16)
        wo3_sb = const.tile([DK, D], BF16)

        kT_s = [const.tile([128, 2, 512], BF16, name=f"kT{i}") for i in range(4)]
        qT_s = [const.tile([128, 2, 512], BF16, name=f"qT{i}") for i in range(4)]
        # v[tt]: [128 kpos, t4, head, dk+ones]
        v_s = [const.tile([128, 4, 4, DK + 1], BF16, name=f"v_{i}") for i in range(4)]
        ctxT_s = [const.tile([128, 2, 512], BF16, name=f"cx{i}") for i in range(4)]

        # ---- input DMAs: 256KB half-row transfers, none on the scalar
        # queue (ACT must not queue exps behind DMAs).  Left halves first
        # so K-st0/st1 and Q0 can start ~8us in.
        # sync queue: wk, xkL, xkR, then xvR (it lands ~3us earlier than
        # at the tail of the Pool queue, unblocking g1's tt2/tt3 V rounds);
        # Pool queue: wq, xq01, wv, xvL, wo, wo3.
        xkL, xkR, xvL, xvR, xq_b, xq23_b = [], [], [], [], [], []
        nc.sync.dma_start(wk_sb[:], wk.rearrange("(i p) c -> p i c", p=128))
        for half, lst in ((0, xkL), (1, xkR)):
            for i in range(8):
                t_ = xkp.tile([128, 1024], BF16, tag="xk", bufs=16, name=f"xk{half}_{i}")
                nc.sync.dma_start(
                    t_[:], xkT[128 * i : 128 * (i + 1), 1024 * half : 1024 * (half + 1)]
                )
                lst.append(t_)
        for i in range(8):
            t_ = xvp.tile([128, 1024], BF16, tag="xv", bufs=16, name=f"xv1_{i}")
            nc.sync.dma_start(t_[:], xvT[128 * i : 128 * (i + 1), 1024:2048])
            xvR.append(t_)
        nc.gpsimd.dma_start(wq_sb[:], wq.rearrange("(i p) c -> p i c", p=128))
        for i in range(8):
            t_ = xqp.tile([128, 1024], BF16, tag="xq", bufs=8, name=f"xq01_{i}")
            nc.gpsimd.dma_start(t_[:], xqT[128 * i : 128 * (i + 1), 0:1024])
            xq_b.append(t_)
        nc.gpsimd.dma_start(wv_sb[:], wv.rearrange("(i p) c -> p i c", p=128))
        for i in range(8):
            t_ = xvp.tile([128, 1024], BF16, tag="xv", bufs=16, name=f"xv0_{i}")
            nc.gpsimd.dma_start(t_[:], xvT[128 * i : 128 * (i + 1), 0:1024])
            xvL.append(t_)
        nc.gpsimd.dma_start(wo_sb[:], wo.rearrange("(c p) o -> p c o", p=128))
        nc.gpsimd.dma_start(wo3_sb[:], wo3[:, :])

        # ones column of V (softmax row sums ride the PV matmul)
        for tt in range(4):
            nc.vector.memset(v_s[tt][:, :, :, DK : DK + 1], 1.0)

        # ---- prefix: K st0/st1 + Q0, i-loop ordered by DMA arrival ----
        # Accumulators: two sc-tag PSUM tiles hold st0/st1 x (cc0|cc1) as
        # 2KB halves (zero regions are 2KB so the interleaved groups are
        # independent); Q0 rides the two pj slots.  One [128,1024] DVE
        # eviction per kT tile.  K st2/st3 and V ride inside g0 as
        # fillers; Q1 inside g1.
        sH = [
            [
                psum.tile([128, 512], F32, tag="sc", bufs=4, name=f"kl{st}{cc}")
                for cc in range(2)
            ]
            for st in range(2)
        ]
        pjQ = [
            psum.tile([128, 512], F32, tag="pj", bufs=2, name=f"q0_{cc}")
            for cc in range(2)
        ]
        for i in range(8):
            st_, sp_ = (i == 0), (i == 7)
            for cc in range(2):
                wsl = wk_sb[:, i, 128 * cc : 128 * (cc + 1)]
                nc.tensor.matmul(
                    sH[0][cc][:], wsl, xkL[i][:, 0:512],
                    start=st_, stop=sp_,
                )
                nc.tensor.matmul(
                    sH[1][cc][:], wsl, xkL[i][:, 512:1024],
                    start=st_, stop=sp_,
                )
            for cc in range(2):
                nc.tensor.matmul(
                    pjQ[cc][:],
                    wq_sb[:, i, 128 * cc : 128 * (cc + 1)],
                    xq_b[i][:, 0:512],
                    start=st_, stop=sp_,
                )
        nc.vector.tensor_copy(kT_s[0][:, 0, :], sH[0][0][:])
        nc.vector.tensor_copy(kT_s[0][:, 1, :], sH[0][1][:])
        nc.scalar.copy(kT_s[1][:, 0, :], sH[1][0][:])
        nc.scalar.copy(kT_s[1][:, 1, :], sH[1][1][:])
        nc.vector.tensor_copy(qT_s[0][:, 0, :], pjQ[0][:])
        nc.scalar.copy(qT_s[0][:, 1, :], pjQ[1][:])

        kr_live = {}

        def kr_mm(st, cc, i):
            # K st2/st3 round, woven into g0 (4 mm/step)
            if i == 0:
                kr_live[(st, cc)] = psum.tile(
                    [128, 512], F32, tag="pj", bufs=2, name=f"kr{st}{cc}"
                )
            ps = kr_live[(st, cc)]
            nc.tensor.matmul(
                ps[:],
                wk_sb[:, i, 128 * cc : 128 * (cc + 1)],
                (xkR[i][:, 0:512] if st == 2 else xkR[i][:, 512:1024]),
                start=(i == 0),
                stop=(i == 7),
            )
            if i == 7:
                ps = kr_live.pop((st, cc))
                if cc == 0:
                    nc.scalar.copy(kT_s[st][:, 0, :], ps[:])
                else:
                    nc.vector.tensor_copy(kT_s[st][:, 1, :], ps[:])

        # ---- attention building blocks --------------------------------
        def scores_pair(qt, p, t):
            scE = psum.tile([128, 512], F32, tag="sc", bufs=4, name=f"sE{qt}{p}{t}")
            scO = psum.tile([128, 512], F32, tag="sc", bufs=4, name=f"sO{qt}{p}{t}")
            kt = kT_s[t // 4]
            ts_ = slice(128 * (t % 4), 128 * (t % 4 + 1))
            qtile = qT_s[qt]
            nc.tensor.matmul(
                scE[:],
                kt[0:64, p, ts_],
                qtile[0:64, p, :],
                start=True,
                stop=True,
                tile_position=(0, 0),
            )
            nc.tensor.matmul(
                scO[:],
                kt[64:128, p, ts_],
                qtile[64:128, p, :],
                start=True,
                stop=True,
                tile_position=(64, 0),
            )
            return scE, scO

        def exp_emit(qt, p, t, scE, scO):
            etE = expp.tile([128, 512], BF16, tag="et", bufs=32, name=f"eE{qt}{p}{t}")
            etO = expp.tile([128, 512], BF16, tag="et", bufs=32, name=f"eO{qt}{p}{t}")
            nc.scalar.activation(etE[:], scE[:], Exp, scale=1.0 / 8.0)
            if t in O_ACT_STEPS:
                nc.scalar.activation(etO[:], scO[:], Exp, scale=1.0 / 8.0)
            else:
                nc.vector._custom_dve(
                    EXP_OP, out=etO[:], in0=scO[:], s0=1.0 / 512.0, s1=1.0
                )
            return etE, etO

        def pv_emit(p, t, et, ctx0, ctx1):
            etE, etO = et
            vg = v_s[t // 4]
            nc.tensor.matmul(
                ctx0[:],
                vg[:, t % 4, 2 * p, :],
                etE[:],
                start=(t == 0),
                stop=(t == 15),
            )
            nc.tensor.matmul(
                ctx1[:],
                vg[:, t % 4, 2 * p + 1, :],
                etO[:],
                start=(t == 0),
                stop=(t == 15),
            )

        def v_round(hp, tt, t4):
            ps = psum.tile([128, 128], F32, tag="pj", bufs=2, name=f"vps{hp}{tt}{t4}")
            xv_half = xvL if tt < 2 else xvR
            c0 = 512 * (tt % 2) + 128 * t4
            for i in range(8):
                nc.tensor.matmul(
                    ps[:],
                    xv_half[i][:, c0 : c0 + 128],
                    wv_sb[:, i, 128 * hp : 128 * (hp + 1)],
                    start=(i == 0),
                    stop=(i == 7),
                )
            dst = v_s[tt][:, t4, 2 * hp : 2 * hp + 2, 0:DK]
            srcv = ps[:].rearrange("p (h d) -> p h d", h=2)
            if t4 % 2 == 0:
                nc.scalar.copy(dst, srcv)
            else:
                nc.vector.tensor_copy(dst, srcv)

        def oblock(qt, o):
            ops = psum.tile([128, 512], F32, tag="pj", bufs=2, name=f"op{qt}_{o}")
            nc.tensor.matmul(
                ops[:],
                wo_sb[:, 0, 128 * o : 128 * (o + 1)],
                ctxT_s[qt][:, 0, :],
                start=True,
                stop=False,
            )
            nc.tensor.matmul(
                ops[:],
                wo_sb[:, 1, 128 * o : 128 * (o + 1)],
                ctxT_s[qt][:, 1, :],
                start=False,
                stop=True,
            )
            ob = smallp.tile([128, 512], BF16, tag="ob", bufs=3)
            if o % 2 == 0:
                nc.scalar.copy(ob[:], ops[:])
            else:
                nc.vector.tensor_copy(ob[:], ops[:])
            nc.sync.dma_start(
                outT[128 * o : 128 * (o + 1), 512 * qt : 512 * (qt + 1)], ob[:]
            )

        ob3_held = {}

        def ob3_c0_open(o):
            # first contraction half, opened during the tail norm window
            ops = psum.tile([128, 512], F32, tag="pj", bufs=2, name=f"o3h{o}")
            nc.tensor.matmul(
                ops[:],
                wo_sb[:, 0, 128 * o : 128 * (o + 1)],
                ctxT_s[3][:, 0, :],
                start=True,
                stop=False,
            )
            ob3_held[o] = ops

        def ob3_block(o):
            # heads 0/1 via K=128 c0; head 2 via K=64 on ctxT even half;
            # head 3 via K=64 on the ct tile through wo3 (no ctxT DMA)
            ops = ob3_held.pop(o, None)
            if ops is None:
                ops = psum.tile([128, 512], F32, tag="pj", bufs=2, name=f"o3{o}")
                nc.tensor.matmul(
                    ops[:],
                    wo_sb[:, 0, 128 * o : 128 * (o + 1)],
                    ctxT_s[3][:, 0, :],
                    start=True,
                    stop=False,
                )
            nc.tensor.matmul(
                ops[:],
                wo_sb[0:DK, 1, 128 * o : 128 * (o + 1)],
                ctxT_s[3][0:DK, 1, :],
                start=False,
                stop=False,
            )
            nc.tensor.matmul(
                ops[:],
                wo3_sb[:, 128 * o : 128 * (o + 1)],
                ct_store["ct3"][:],
                start=False,
                stop=True,
            )
            ob = smallp.tile([128, 512], BF16, tag="ob", bufs=3)
            if o % 2 == 0:
                nc.scalar.copy(ob[:], ops[:])
            else:
                nc.vector.tensor_copy(ob[:], ops[:])
            nc.sync.dma_start(
                outT[128 * o : 128 * (o + 1), 512 * 3 : 512 * 4], ob[:]
            )

        # Q proj for qt 1/2/3, spread across steps (pj slot held across)
        qproj_live = {}

        def qproj_mm(qtn, cc, i):
            if i == 0:
                qproj_live[(qtn, cc)] = psum.tile(
                    [128, 512], F32, tag="pj", bufs=2, name=f"qp{qtn}{cc}"
                )
            ps = qproj_live[(qtn, cc)]
            xsrc = (
                xq_b[i][:, 512:1024]
                if qtn == 1
                else xq23_b[i][:, 512 * (qtn - 2) : 512 * (qtn - 1)]
            )
            nc.tensor.matmul(
                ps[:],
                wq_sb[:, i, 128 * cc : 128 * (cc + 1)],
                xsrc,
                start=(i == 0),
                stop=(i == 7),
            )
            if i == 7:
                nc.scalar.copy(qT_s[qtn][:, cc, :], qproj_live.pop((qtn, cc)))

        def xq_load(qtn):
            if qtn != 2:
                return
            for i in range(8):
                t_ = xqp.tile([128, 1024], BF16, tag="xq", bufs=8, name=f"xq23_{i}")
                nc.sync.dma_start(t_[:], xqT[128 * i : 128 * (i + 1), 1024:2048])
                xq23_b.append(t_)

        def mk_norm(cxE, cxO, qt, p):
            # both rowsums sit at psum row 64; stage them to partition 0
            # (proven cross-partition [1,512] copies), one fused reciprocal
            rs = smallp.tile([1, 2, 512], F32, tag="rs1", bufs=2)
            nc.vector.tensor_copy(rs[:, 0, :], cxE[DK : DK + 1, :])
            nc.vector.tensor_copy(rs[:, 1, :], cxO[DK : DK + 1, :])
            rc = smallp.tile([1, 2, 512], F32, tag="rc", bufs=2)
            nc.vector.reciprocal_approx_fast(rc[:], rs[:])
            bcE = smallp.tile([64, 512], F32, tag="bcE", bufs=2)
            nc.gpsimd.partition_broadcast(bcE[:], rc[:, 0, :])
            bcO = smallp.tile([64, 512], F32, tag="bcO", bufs=2)
            nc.gpsimd.partition_broadcast(bcO[:], rc[:, 1, :])
            nc.vector.tensor_tensor(
                ctxT_s[qt][0:DK, p, :], cxE[0:DK, :], bcE[:], MULT
            )
            ct = smallp.tile([64, 512], BF16, tag="ct", bufs=2)
            nc.vector.tensor_tensor(ct[:], cxO[0:DK, :], bcO[:], MULT)
            nc.sync.dma_start(ctxT_s[qt][DK : 2 * DK, p, :], ct[:])

        # ---- attention groups -----------------------------------------
        # pending: work carried into the next group's first steps
        pending = []

        def drain_pending(upto):
            while pending and pending[0][0] <= upto:
                pending.pop(0)[1]()

        def emit_group(qt, p):
            gi = 2 * qt + p
            ctx0 = psum.tile([DK + 1, 512], F32, tag="ctxE", bufs=1, name=f"cx{qt}{p}0")
            ctx1 = psum.tile([DK + 1, 512], F32, tag="ctxO", bufs=1, name=f"cx{qt}{p}1")
            ets = {}

            # per-step pv emission plan (lagged so the PE never waits on exp,
            # and so every v_round/kr_mm a pv needs precedes it in the queue)
            if gi == 0:
                pv_plan = {10: [0, 1], 11: [2, 3], 12: [4, 5], 13: [6, 7, 8],
                           14: [9, 10, 11], 15: [12, 13]}
            elif gi == 1:
                pv_plan = {8: [0, 1], 9: [2, 3], 10: [4, 5], 11: [6, 7],
                           12: [8, 9], 13: [10, 11], 14: [12, 13]}
            else:
                pv_plan = {t: [t - 2] for t in range(2, 16)}

            # per-step filler plan
            fillers = {t: [] for t in range(16)}
            if gi == 0:
                # K st2/st3 (4 mm/step, steps 0-7), V heads 0/1 (2/step, 8-15)
                for j in range(32):
                    st, cc, i = 2 + j // 16, (j % 16) // 8, j % 8
                    fillers[2 + j // 4].append(lambda st=st, cc=cc, i=i: kr_mm(st, cc, i))
                for j in range(16):
                    fillers[8 + j // 2].append(
                        lambda j=j: v_round(0, j // 4, j % 4)
                    )
            elif gi == 1:
                # V heads 2/3 (2/step, steps 0-7), Q1 (2 mm/step, 8-15)
                for j in range(16):
                    fillers[j // 2].append(lambda j=j: v_round(1, j // 4, j % 4))
                for j in range(16):
                    cc, i = j // 8, j % 8
                    fillers[8 + j // 2].append(lambda cc=cc, i=i: qproj_mm(1, cc, i))
            else:
                if gi in (2, 4):
                    fillers[2].append(lambda qtn=qt + 1: xq_load(qtn))
                if gi in (2, 3, 4, 5):
                    qtn, cc = qt + 1, p
                    i = 0
                    for t_, n_ in ((10, 1), (11, 1), (12, 1), (13, 1), (14, 2), (15, 2)):
                        for _ in range(n_):
                            fillers[t_].append(
                                lambda qtn=qtn, cc=cc, i=i: qproj_mm(qtn, cc, i)
                            )
                            i += 1
                ob_base = 4 * p
                for j, t in enumerate((3, 5, 7, 9)):
                    fillers[t].append(lambda qt=qt, o=ob_base + j: oblock(qt - 1, o))
                if gi == 7:
                    for j, t in enumerate((10, 12, 13, 15)):
                        fillers[t].append(lambda o=j: ob3_partial_c0(o))

            for t in range(16):
                if t == 0:
                    drain_pending(0)
                sc = scores_pair(qt, p, t)
                ets[t] = exp_emit(qt, p, t, sc)
                if t == 1:
                    drain_pending(1)
                for tp in pv_plan.get(t, ()):
                    pv_emit(p, tp, ets.pop(tp), ctx0, ctx1)
                for f in fillers[t]:
                    f()

            # carry the drain into the next group
            def fin(qt=qt, p=p, ctx0=ctx0, ctx1=ctx1, ets=ets):
                pv_emit(p, 14, ets.pop(14), ctx0, ctx1)
                pv_emit(p, 15, ets.pop(15), ctx0, ctx1)
                mk_norm(ctx0, ctx1, qt, p)

            if qt == 3 and p == 1:
                fin()
            else:
                pending.append((0, fin))

        for qt in range(4):
            for p in range(2):
                emit_group(qt, p)

        # ---- tail: qt3 output projection.  c0 of blocks 0/1 opens during
        # the norm-chain window; every block then adds head 2 from the even
        # ctxT half and head 3 straight from the ct tile via wo3, so nothing
        # waits on a ctxT DMA.
        ob3_c0_open(0)
        ob3_c0_open(1)
        for o in range(8):
            ob3_block(o)

    nc.finalize()
    return nc


def _get_nc():
    if "nc" not in _CACHE:
        _CACHE["nc"] = _build()
    return _CACHE["nc"]


def _numpy_reference(query, key, value, mask, Wq, Aq, Bq, Wk, Ak, Bk, Wv, Av, Bv, Wo, Ao, Bo):
    """Exact fallback for a non-all-ones mask (never hit for the spec'd inputs)."""

    def lora(x, W, A, Bm):
        return x @ W.T + ((x @ A.T) @ Bm.T) * SCALING

    q = lora(query, Wq, Aq, Bq).reshape(B, S, H, DK).transpose(0, 2, 1, 3)
    k = lora(key, Wk, Ak, Bk).reshape(B, S, H, DK).transpose(0, 2, 1, 3)
    v = lora(value, Wv, Av, Bv).reshape(B, S, H, DK).transpose(0, 2, 1, 3)
    sc = np.einsum("bhqd,bhkd->bhqk", q, k) / np.sqrt(np.float32(DK))
    sc = np.where(mask == 0, np.float32(-1e9), sc)
    sc = sc - sc.max(axis=-1, keepdims=True)
    e = np.exp(sc)
    attn = e / e.sum(axis=-1, keepdims=True)
    cx = np.einsum("bhqk,bhkd->bhqd", attn, v)
    cx = cx.transpose(0, 2, 1, 3).reshape(B, S, D)
    return lora(cx, Wo, Ao, Bo).astype(np.float32)


def _prepare_in_maps(query, key, value, Wq, Aq, Bq, Wk, Ak, Bk, Wv, Av, Bv, Wo, Ao, Bo):
    f32 = np.float32
    bf16 = ml_dtypes.bfloat16
    weff = {}
    for n, (W, A, Bm) in {
        "q": (Wq, Aq, Bq),
        "k": (Wk, Ak, Bk),
        "v": (Wv, Av, Bv),
        "o": (Wo, Ao, Bo),
    }.items():
        weff[n] = (
            np.asarray(W, f32) + SCALING * np.asarray(Bm, f32) @ np.asarray(A, f32)
        ).astype(f32)

    xT = {
        "q": [np.ascontiguousarray(np.asarray(query[b], f32).T).astype(bf16) for b in range(B)],
        "k": [np.ascontiguousarray(np.asarray(key[b], f32).T).astype(bf16) for b in range(B)],
        "v": [np.ascontiguousarray(np.asarray(value[b], f32).T).astype(bf16) for b in range(B)],
    }
    in_maps = []
    for c in range(N_CORES):
        b, g = divmod(c, CPG)
        cs = slice(CSLICE * g, CSLICE * (g + 1))
        wo_arr = np.ascontiguousarray(weff["o"][:, cs].T).astype(bf16)
        in_maps.append(
            {
                "xqT": xT["q"][b],
                "xkT": xT["k"][b],
                "xvT": xT["v"][b],
                "wq": np.ascontiguousarray(weff["q"][cs, :].T).astype(bf16),
                "wk": np.ascontiguousarray(weff["k"][cs, :].T).astype(bf16),
                "wv": np.ascontiguousarray(weff["v"][cs, :].T).astype(bf16),
                "wo": wo_arr,
                "wo3": np.ascontiguousarray(wo_arr[192:256, :]),
            }
        )
    return in_maps


def run(inputs, trace=False, **spmd_kwargs):
    """Shard, run on 8 cores, gather.  Returns (output, BassKernelResults)."""
    mask = np.asarray(inputs["mask"])
    if not np.all(mask != 0):
        out = _numpy_reference(
            np.asarray(inputs["query"], np.float32),
            np.asarray(inputs["key"], np.float32),
            np.asarray(inputs["value"], np.float32),
            mask,
            *[
                np.asarray(inputs[k], np.float32)
                for k in ("Wq", "Aq", "Bq", "Wk", "Ak", "Bk", "Wv", "Av", "Bv", "Wo", "Ao", "Bo")
            ],
        )
        return out, None

    in_maps = _prepare_in_maps(
        inputs["query"], inputs["key"], inputs["value"],
        inputs["Wq"], inputs["Aq"], inputs["Bq"],
        inputs["Wk"], inputs["Ak"], inputs["Bk"],
        inputs["Wv"], inputs["Av"], inputs["Bv"],
        inputs["Wo"], inputs["Ao"], inputs["Bo"],
    )
    nc = _get_nc()
    res = run_bass_kernel_spmd(
        nc, in_maps, core_ids=list(range(N_CORES)), trace=trace, **spmd_kwargs
    )
    out = np.empty((B, S, D), np.float32)
    for b in range(B):
        acc = res.results[CPG * b]["outT"].astype(np.float32)
        for g in range(1, CPG):
            acc = acc + res.results[CPG * b + g]["outT"].astype(np.float32)
        out[b] = acc.T
    return out, res


def kernel(**inputs):
    out, _ = run(inputs, trace=False)
    return out

